# revision 55
# baseline (speedup 1.0000x reference)
"""Trainium2 Bass kernel for nn_Attention (B=16, N=1024, C=1024, H=16, pre-LN +
q/k post-LN attention block), data-parallel over 8 NeuronCores (2 batches/core).

v2 design (vs v1 baseline):
  - all-bf16 datapath (weights, y, q, k, v, e, AO); fp32 PSUM accumulate.
  - weights loaded into SBUF once per kernel (not per batch), in 8 chunk DMAs
    each, interleaved with the first batch's LayerNorm.
  - pre-LN y transposed to yT via DMA xbar transposes (no PE/PSUM involved).
  - N=1024 moving operands everywhere (bf16 allows 1024-wide matmuls).
  - exp tiles are [128 kt, 1024 qt]; the k-head post-LN rstd is folded into
    the exp per-partition `scale` operand (eliminates k's broadcast matmul
    and multiply). q keeps the broadcast-matmul scaling (folds the 1/8
    attention scale via the 64*eps trick).
  - softmax normalization deferred: stage B emits raw O rows + denominator
    reciprocals; a later "sweep" (broadcast matmul + in-place multiply)
    normalizes AO. Output projection bias added by DVE from a broadcast tile.
  - software pipeline: B(b0) overlaps A1(b1) + weight loads; B(b1) overlaps
    sweep+C(b0); all stages software-pipelined to keep PE/Act dense.
"""

import numpy as np

B, N, C, H, Dh = 16, 1024, 1024, 16, 64
NCORES = 8
BL = B // NCORES          # batches per core
T = BL * N                # tokens per core
CCH = C // 128            # contraction chunks
NB = N // 128             # token tiles per batch
EPS = 1e-6

_cache: dict = {}


def _build():
    from contextlib import ExitStack

    import concourse.bacc as bacc
    import concourse.mybir as mybir
    import concourse.tile as tile

    F32 = mybir.dt.float32
    F32R = mybir.dt.float32r
    BF16 = mybir.dt.bfloat16
    AF = mybir.ActivationFunctionType
    OP = mybir.AluOpType

    nc = bacc.Bacc("TRN2", target_bir_lowering=False, debug=False,
                   num_devices=NCORES)

    x_d = nc.dram_tensor("xbf", [T, C], BF16, kind="ExternalInput").ap()
    wqt_d = nc.dram_tensor("wqt", [C, C], BF16, kind="ExternalInput").ap()
    wkt_d = nc.dram_tensor("wkt", [C, C], BF16, kind="ExternalInput").ap()
    wvt_d = nc.dram_tensor("wvt", [C, C], BF16, kind="ExternalInput").ap()
    wpt_d = nc.dram_tensor("wpt", [C, C], BF16, kind="ExternalInput").ap()
    bpb_d = nc.dram_tensor("bpb", [128, C], BF16, kind="ExternalInput").ap()
    csel_d = nc.dram_tensor("c_sel", [128, 128], BF16,
                            kind="ExternalInput").ap()
    ce2_d = nc.dram_tensor("c_e2", [128, 2], BF16, kind="ExternalInput").ap()
    cb2_d = nc.dram_tensor("c_b2", [2, 128], BF16, kind="ExternalInput").ap()
    ceps_d = nc.dram_tensor("c_eps", [128, 2], F32, kind="ExternalInput").ap()
    out_d = nc.dram_tensor("out", [T, C], F32, kind="ExternalOutput").ap()

    with tile.TileContext(nc) as tc, ExitStack() as top:
        # ---- constants ----
        const = top.enter_context(tc.tile_pool(name="const", bufs=1))
        e2 = const.tile([128, 2], BF16)
        nc.gpsimd.dma_start(out=e2, in_=ce2_d)
        b2 = const.tile([2, 128], BF16)
        nc.gpsimd.dma_start(out=b2, in_=cb2_d)
        ceps = const.tile([128, 2], F32)
        nc.gpsimd.dma_start(out=ceps, in_=ceps_d)
        eps_t = ceps[:, 0:1]
        eps64_t = ceps[:, 1:2]
        bpb = const.tile([128, C], BF16)
        nc.gpsimd.dma_start(out=bpb, in_=bpb_d)
        selc = const.tile([128, 128], BF16)
        nc.gpsimd.dma_start(out=selc, in_=csel_d)

        wpool = top.enter_context(tc.tile_pool(name="w", bufs=1))
        wts = {k: wpool.tile([128, CCH, C], BF16, name=f"w_{k}")
               for k in ("v", "q", "k", "p")}

        def w_dma_units(kinds):
            for k, dram in kinds:
                dr = dram.rearrange("(cc p) d -> p cc d", p=128)
                for cc in range(CCH):
                    def u(k=k, dr=dr, cc=cc):
                        nc.gpsimd.dma_start(out=wts[k][:, cc, :],
                                            in_=dr[:, cc, :])
                    yield u

        # ---- persistent tiles; yT/qkv lifetimes are disjoint across the two
        # batches so one set is shared; AO/rA overlap in P4 so two copies ----
        big = top.enter_context(tc.tile_pool(name="big", bufs=1))
        shared_tiles = dict(
            yT=big.tile([128, CCH, N], BF16, name="yT"),
            qT=big.tile([128, CCH, N], BF16, name="qT"),
            kT=big.tile([128, CCH, N], BF16, name="kT"),
        )
        bstate = []
        for b in range(BL):
            st = dict(shared_tiles)
            st["vS"] = big.tile([128, NB, H, Dh + 1], BF16, name=f"vS{b}")
            st["AO"] = big.tile([128, CCH, N], BF16, name=f"AO{b}")
            st["stg"] = big.tile([128, N], BF16, name=f"stg{b}")
            bstate.append(st)

        # ================= phase emitters =================

        def a1_units(b, ph, ph_inner=None):
            """Token centering y = x - mean(x) + DMA-transpose to yT. The
            pre-LN rstd is NOT applied to y: the per-head post-LN on q/k is
            scale-invariant per token (weights are centered), so only v needs
            it — folded into the v psum->SBUF copy as a per-partition scale.
            Act sqrts happen in one burst to avoid act-table thrash against
            the exp stream. Returns (part1s, [sqrt burst], rstd_fn)."""
            st = bstate[b]
            phi = ph_inner if ph_inner is not None else ph
            mp = ph.enter_context(tc.tile_pool(name=f"a1m{b}", bufs=1))
            mvAll = mp.tile([128, NB, nc.vector.BN_AGGR_DIM], F32,
                            name=f"mv{b}")
            rsAll = mp.tile([128, NB, 1], F32, name=f"rs{b}")
            xp = phi.enter_context(tc.tile_pool(name=f"a1x{b}", bufs=3))
            sp = phi.enter_context(tc.tile_pool(name=f"a1s{b}", bufs=3))
            yp = phi.enter_context(tc.tile_pool(name=f"a1y{b}", bufs=2))
            p1s = []
            for t in range(NB):
                def part1(t=t):
                    r0 = b * N + t * 128
                    xt = xp.tile([128, C], BF16, tag="xt")
                    nc.sync.dma_start(out=xt, in_=x_d[r0:r0 + 128, :])
                    stats = sp.tile([128, 2, nc.vector.BN_STATS_DIM], F32,
                                    tag="st")
                    xg = xt.rearrange("p (s f) -> p s f", s=2)
                    for s in range(2):
                        nc.vector.bn_stats(out=stats[:, s, :], in_=xg[:, s, :])
                    nc.vector.bn_aggr(out=mvAll[:, t, :], in_=stats)
                    y = yp.tile([128, C], BF16, tag="y")
                    nc.vector.tensor_scalar(
                        out=y, in0=xt, scalar1=mvAll[:, t, 0:1], scalar2=None,
                        op0=OP.subtract)
                    nc.scalar.dma_start(
                        out=st["yT"][:, :, t * 128:(t + 1) * 128], in_=y,
                        transpose=True)

                p1s.append(part1)

            def sqrt_burst():
                # single Act instruction (cannot be scheduler-split between
                # exps) + single DVE reciprocal for all NB tiles' rstd
                nc.scalar.activation(rsAll, mvAll[:, :, 1:2], AF.Sqrt,
                                     bias=eps_t)
                nc.vector.reciprocal_approx_fast(out=rsAll, in_=rsAll)

            def rstd_of(t):
                return rsAll[:, t, 0:1]

            return p1s, [sqrt_burst], rstd_of

        def v_units(b, pool, rstd_of):
            """v projection units (one per token tile), using psum `pool`.
            Applies the pre-LN rstd as a per-partition (per-token) scale."""
            st = bstate[b]
            units = []
            for tt in range(NB):
                def vunit(tt=tt):
                    for d2 in range(2):
                        ps = pool.tile([128, 512], F32, tag="ppv", name="ppv")
                        for cc in range(CCH):
                            nc.tensor.matmul(
                                ps, st["yT"][:, cc, tt * 128:(tt + 1) * 128],
                                wts["v"][:, cc, d2 * 512:(d2 + 1) * 512],
                                start=(cc == 0), stop=(cc == CCH - 1))
                        nc.vector.tensor_scalar(
                            out=st["vS"][:, tt, d2 * 8:(d2 + 1) * 8, 0:Dh],
                            in0=ps.rearrange("p (h e) -> p h e", e=Dh),
                            scalar1=rstd_of(tt), scalar2=None, op0=OP.mult)
                    nc.gpsimd.memset(st["vS"][:, tt, :, Dh:Dh + 1], 1.0)
                units.append(vunit)
            return units

        def pipeline_units(*stages):
            """Yield stage-s unit j at step j+s (software pipeline)."""
            n = max(len(s) for s in stages)
            for i in range(n + len(stages) - 1):
                for s, lst in enumerate(stages):
                    j = i - s
                    if 0 <= j < len(lst):
                        yield lst[j]

        def a2_units(b, ph, rstd_of=None):
            """q/k (+v when rstd_of given) projections, software-pipelined."""
            st = bstate[b]
            pp = ph.enter_context(tc.tile_pool(name=f"pp{b}", bufs=2,
                                               space="PSUM"))
            ppv = ph.enter_context(tc.tile_pool(name=f"pv{b}", bufs=1,
                                                space="PSUM"))
            sbq = ph.enter_context(tc.tile_pool(name=f"sq{b}", bufs=1,
                                                space="PSUM"))
            sb = ph.enter_context(tc.tile_pool(name=f"sb{b}", bufs=1,
                                               space="PSUM"))
            wkk = ph.enter_context(tc.tile_pool(name=f"wk{b}", bufs=2))
            wk2 = ph.enter_context(tc.tile_pool(name=f"w2{b}", bufs=1))

            if rstd_of is not None:
                for u in v_units(b, ppv, rstd_of):
                    yield u

            # q then k, per head-pair chunk dc; parts pipelined at depth 2
            specs = [(0, dc) for dc in range(CCH)] + \
                    [(1, dc) for dc in range(CCH)]
            states = [{} for _ in specs]

            def partA(u):
                wi, dc = specs[u]
                s = states[u]
                wt = wts["q"] if wi == 0 else wts["k"]
                raw = (st["qT"] if wi == 0 else st["kT"])[:, dc, :]
                ps = pp.tile([128, N], F32, tag="pp")
                for q2 in range(2):
                    sl = slice(q2 * 512, (q2 + 1) * 512)
                    for cc in range(CCH):
                        nc.tensor.matmul(
                            ps[:, sl], wt[:, cc, dc * 128:(dc + 1) * 128],
                            st["yT"][:, cc, sl],
                            start=(cc == 0), stop=(cc == CCH - 1))
                nc.vector.tensor_copy(out=raw, in_=ps)
                sq = wkk.tile([128, N], BF16, tag="sq")
                nc.vector.tensor_mul(sq, raw, raw)
                s["raw"], s["sq"] = raw, sq

            def partB(u):
                wi, dc = specs[u]
                s = states[u]
                ssq = sbq.tile([2, N], F32, tag="ssq")
                for q2 in range(2):
                    sl = slice(q2 * 512, (q2 + 1) * 512)
                    nc.tensor.matmul(ssq[:, sl], e2, s["sq"][:, sl],
                                     start=True, stop=True)
                stdt = wk2.tile([2, N], F32, tag="stdt")
                if wi == 0:
                    # 0.125/sqrt(ssq/64+eps) == 1/sqrt(ssq+64eps)
                    nc.scalar.activation(stdt, ssq, AF.Sqrt,
                                         bias=eps64_t[0:2, :])
                else:
                    nc.scalar.activation(stdt, ssq, AF.Sqrt,
                                         bias=eps_t[0:2, :], scale=1.0 / 64.0)
                rst = wk2.tile([2, N], BF16, tag="rst")
                with nc.allow_low_precision(reason="bf16 rstd"):
                    nc.vector.reciprocal(rst, stdt)
                s["rst"] = rst

            def partC(u):
                wi, dc = specs[u]
                s = states[u]
                for q2 in range(2):
                    sl = slice(q2 * 512, (q2 + 1) * 512)
                    bc = sb.tile([128, 512], F32, tag="bc", name="bc")
                    nc.tensor.matmul(bc, b2, s["rst"][:, sl],
                                     start=True, stop=True)
                    nc.vector.tensor_tensor(out=s["raw"][:, sl],
                                            in0=s["raw"][:, sl],
                                            in1=bc, op=OP.mult)

            nu = len(specs)
            for u in range(nu + 2):
                def unit(u=u):
                    if u < nu:
                        partA(u)
                    if 1 <= u < nu + 1:
                        partB(u - 1)
                    if 2 <= u:
                        partC(u - 2)
                yield unit


        def b_units(b, ph, o_bufs):
            """Attention stage: one flat software-pipelined stream over all
            (head, kc) steps — s/e of step i+1 always precede o of step i in
            program order, across head boundaries, so the exp stream never
            drains."""
            st = bstate[b]
            spool = ph.enter_context(tc.tile_pool(name=f"s{b}", bufs=2,
                                                  space="PSUM"))
            opool = ph.enter_context(tc.tile_pool(name=f"o{b}", bufs=o_bufs,
                                                  space="PSUM"))
            epool = ph.enter_context(tc.tile_pool(name=f"e{b}", bufs=2))
            bcp = ph.enter_context(tc.tile_pool(name=f"bc{b}", bufs=1,
                                                space="PSUM"))
            nc.gpsimd.memset(st["stg"][64:128, :], 0.0)
            steps = [(h, kc) for h in range(H) for kc in range(NB)]
            state = {}

            def se(i):
                h, kc = steps[i]
                hp, par = h // 2, h % 2
                s = spool.tile([128, N], F32, tag="s")
                for q2 in range(2):
                    sl = slice(q2 * 512, (q2 + 1) * 512)
                    nc.tensor.matmul(
                        s[:, sl], st["kT"][64 * par:64 * par + 64, hp,
                                           kc * 128:(kc + 1) * 128],
                        st["qT"][64 * par:64 * par + 64, hp, sl],
                        start=True, stop=True)
                e = epool.tile([128, N], BF16, tag="e")
                nc.scalar.activation(e, s, AF.Exp)
                state[i] = e

            def ov(i):
                h, kc = steps[i]
                hp, par = h // 2, h % 2
                if kc == 0:
                    opsum = opool.tile([Dh + 1, N], F32, tag="o", name="o")
                    state["o"] = opsum
                e = state.pop(i)
                for q2 in range(2):
                    sl = slice(q2 * 512, (q2 + 1) * 512)
                    nc.tensor.matmul(state["o"][:, sl],
                                     st["vS"][:, kc, h, :], e[:, sl],
                                     start=(kc == 0), stop=(kc == NB - 1))
                if kc == NB - 1:
                    opsum = state["o"]
                    # denominator reciprocal into stage row 64 (even head)
                    # or 96 (odd head) - 32-aligned partitions
                    row = 64 + 32 * par
                    with nc.allow_low_precision(reason="bf16 denom"):
                        nc.vector.reciprocal(st["stg"][row:row + 1, :],
                                             opsum[Dh:Dh + 1, :])
                    nc.vector.tensor_copy(
                        out=st["AO"][64 * par:64 * par + 64, hp, :],
                        in_=opsum[0:Dh, :])
                    if par == 1:
                        # normalize the completed head pair: broadcast the
                        # two recip rows to 128 partitions, scale AO in place
                        for q2 in range(2):
                            sl = slice(q2 * 512, (q2 + 1) * 512)
                            bc = bcp.tile([128, 512], F32, tag="bc",
                                          name="bc")
                            nc.tensor.matmul(
                                bc, selc[64:128, :], st["stg"][64:128, sl],
                                start=True, stop=True)
                            nc.vector.tensor_tensor(
                                out=st["AO"][:, hp, sl],
                                in0=st["AO"][:, hp, sl], in1=bc, op=OP.mult)

            ns = len(steps)
            for i in range(ns + 1):
                def unit(i=i):
                    if i < ns:
                        se(i)
                    if i >= 1:
                        ov(i - 1)
                yield unit

        def c_units(b, ph, bufs):
            """Output projection; one unit per (tt, d2)."""
            st = bstate[b]
            cps = ph.enter_context(tc.tile_pool(name=f"cp{b}", bufs=bufs,
                                                space="PSUM"))
            cop = ph.enter_context(tc.tile_pool(name=f"co{b}", bufs=3))
            for tt in range(NB):
                for d2 in range(2):
                    def unit(tt=tt, d2=d2):
                        ps = cps.tile([128, 512], F32, tag="cp")
                        for cc in range(CCH):
                            nc.tensor.matmul(
                                ps, st["AO"][:, cc, tt * 128:(tt + 1) * 128],
                                wts["p"][:, cc, d2 * 512:(d2 + 1) * 512],
                                start=(cc == 0), stop=(cc == CCH - 1))
                        osb = cop.tile([128, 512], F32, tag="osb")
                        nc.vector.tensor_tensor(
                            out=osb, in0=ps,
                            in1=bpb[:, d2 * 512:(d2 + 1) * 512], op=OP.add)
                        nc.sync.dma_start(
                            out=out_d[b * N + tt * 128:b * N + (tt + 1) * 128,
                                      d2 * 512:(d2 + 1) * 512],
                            in_=osb)
                    yield unit

        def run_all(gen):
            for u in gen:
                u()

        def run_interleaved(main_gen, fill_gen, fill_per_main):
            fill_iter = iter(fill_gen)
            acc = 0.0
            for u in main_gen:
                u()
                acc += fill_per_main
                while acc >= 1.0:
                    acc -= 1.0
                    done = True
                    for f in fill_iter:
                        f()
                        done = False
                        break
                    if done:
                        acc = 0.0
            for f in fill_iter:
                f()

        # ================= schedule =================
        with ExitStack() as ph01:
            # P0: A1(b0), weight loads interleaved into the DMA queue
            with ExitStack() as ph0i:
                p1s, burst, rstd0 = a1_units(0, ph01, ph0i)
                run_interleaved(iter(p1s + burst),
                                w_dma_units([("v", wvt_d), ("q", wqt_d),
                                             ("k", wkt_d)]), 2.0)
                run_all(w_dma_units([("p", wpt_d)]))
            run_all(a2_units(0, ph01, rstd_of=rstd0))      # P1
        with ExitStack() as ph2:           # P2: B(0) || A1(1)+v(1)
            ppv = ph2.enter_context(tc.tile_pool(name="ppv", bufs=1,
                                                 space="PSUM"))
            p1s, burst, rstd1 = a1_units(1, ph2)
            fills = p1s + burst + v_units(1, ppv, rstd1)
            run_interleaved(b_units(0, ph2, o_bufs=1), iter(fills), 0.14)
        with ExitStack() as ph3:
            run_all(a2_units(1, ph3))                      # P3
        with ExitStack() as ph4:           # P4: B(1) || C(0)
            run_interleaved(b_units(1, ph4, o_bufs=1),
                            iter(c_units(0, ph4, bufs=1)), 0.13)
        with ExitStack() as ph5:           # P5: C(1)
            run_all(c_units(1, ph5, bufs=2))

    nc.compile()
    return nc


def _get_nc():
    if "nc" not in _cache:
        _cache["nc"] = _build()
    return _cache["nc"]


def _host_inputs(Wq, Wk, Wv, Wp, bp):
    """Shared (core-independent) derived weight tensors."""
    import ml_dtypes
    bf16 = ml_dtypes.bfloat16

    def center(Wm):
        Wh = np.asarray(Wm, dtype=np.float32).reshape(H, Dh, C)
        return (Wh - Wh.mean(axis=1, keepdims=True)).reshape(C, C)

    e2 = np.zeros((128, 2), np.float32)
    e2[0:64, 0] = 1.0
    e2[64:128, 1] = 1.0
    b2 = np.zeros((2, 128), np.float32)
    b2[0, 0:64] = 1.0
    b2[1, 64:128] = 1.0
    eps = np.zeros((128, 2), np.float32)
    eps[:, 0] = EPS
    eps[:, 1] = 64.0 * EPS
    bpb = np.broadcast_to(np.asarray(bp, np.float32).reshape(1, C),
                          (128, C)).astype(bf16)
    sel = np.zeros((128, 128), np.float32)
    sel[64, 0:64] = 1.0
    sel[96, 64:128] = 1.0
    return {
        "c_sel": sel.astype(bf16),
        "c_e2": e2.astype(bf16),
        "c_b2": b2.astype(bf16),
        "c_eps": eps,
        "bpb": bpb,
        "wqt": np.ascontiguousarray(center(Wq).T).astype(bf16),
        "wkt": np.ascontiguousarray(center(Wk).T).astype(bf16),
        "wvt": np.ascontiguousarray(np.asarray(Wv, np.float32).T).astype(bf16),
        "wpt": np.ascontiguousarray(np.asarray(Wp, np.float32).T).astype(bf16),
    }


def _in_maps(x, Wq, Wk, Wv, Wp, bp):
    import ml_dtypes

    shared = _host_inputs(Wq, Wk, Wv, Wp, bp)
    xbf = np.asarray(x, dtype=np.float32).astype(ml_dtypes.bfloat16)
    return [
        dict(shared,
             xbf=np.ascontiguousarray(xbf[c * BL:(c + 1) * BL].reshape(T, C)))
        for c in range(NCORES)
    ]


def kernel(x, Wq, Wk, Wv, Wp, bp):
    from concourse.bass_utils import run_bass_kernel_spmd

    nc = _get_nc()
    in_maps = _in_maps(x, Wq, Wk, Wv, Wp, bp)
    res = run_bass_kernel_spmd(nc, in_maps, core_ids=list(range(NCORES)))
    out = np.stack([res.results[c]["out"].reshape(BL, N, C)
                    for c in range(NCORES)])
    return out.reshape(B, N, C).astype(np.float32)


# revision 56
# speedup vs baseline: 1.0110x; 1.0110x over previous
"""Trainium2 Bass kernel for nn_Attention (B=16, N=1024, C=1024, H=16, pre-LN +
q/k post-LN attention block), data-parallel over 8 NeuronCores (2 batches/core).

v2 design (vs v1 baseline):
  - all-bf16 datapath (weights, y, q, k, v, e, AO); fp32 PSUM accumulate.
  - weights loaded into SBUF once per kernel (not per batch), in 8 chunk DMAs
    each, interleaved with the first batch's LayerNorm.
  - pre-LN y transposed to yT via DMA xbar transposes (no PE/PSUM involved).
  - N=1024 moving operands everywhere (bf16 allows 1024-wide matmuls).
  - exp tiles are [128 kt, 1024 qt]; the k-head post-LN rstd is folded into
    the exp per-partition `scale` operand (eliminates k's broadcast matmul
    and multiply). q keeps the broadcast-matmul scaling (folds the 1/8
    attention scale via the 64*eps trick).
  - softmax normalization deferred: stage B emits raw O rows + denominator
    reciprocals; a later "sweep" (broadcast matmul + in-place multiply)
    normalizes AO. Output projection bias added by DVE from a broadcast tile.
  - software pipeline: B(b0) overlaps A1(b1) + weight loads; B(b1) overlaps
    sweep+C(b0); all stages software-pipelined to keep PE/Act dense.
"""

import numpy as np

B, N, C, H, Dh = 16, 1024, 1024, 16, 64
NCORES = 8
BL = B // NCORES          # batches per core
T = BL * N                # tokens per core
CCH = C // 128            # contraction chunks
NB = N // 128             # token tiles per batch
EPS = 1e-6

_cache: dict = {}


def _build():
    from contextlib import ExitStack

    import concourse.bacc as bacc
    import concourse.mybir as mybir
    import concourse.tile as tile

    F32 = mybir.dt.float32
    F32R = mybir.dt.float32r
    BF16 = mybir.dt.bfloat16
    AF = mybir.ActivationFunctionType
    OP = mybir.AluOpType

    nc = bacc.Bacc("TRN2", target_bir_lowering=False, debug=False,
                   num_devices=NCORES)

    x_d = nc.dram_tensor("xbf", [T, C], BF16, kind="ExternalInput").ap()
    wqt_d = nc.dram_tensor("wqt", [C, C], BF16, kind="ExternalInput").ap()
    wkt_d = nc.dram_tensor("wkt", [C, C], BF16, kind="ExternalInput").ap()
    wvt_d = nc.dram_tensor("wvt", [C, C], BF16, kind="ExternalInput").ap()
    wpt_d = nc.dram_tensor("wpt", [C, C], BF16, kind="ExternalInput").ap()
    bpb_d = nc.dram_tensor("bpb", [128, C], BF16, kind="ExternalInput").ap()
    csel_d = nc.dram_tensor("c_sel", [128, 128], BF16,
                            kind="ExternalInput").ap()
    ce2_d = nc.dram_tensor("c_e2", [128, 2], BF16, kind="ExternalInput").ap()
    cb2_d = nc.dram_tensor("c_b2", [2, 128], BF16, kind="ExternalInput").ap()
    ceps_d = nc.dram_tensor("c_eps", [128, 2], F32, kind="ExternalInput").ap()
    out_d = nc.dram_tensor("out", [T, C], BF16, kind="ExternalOutput").ap()

    with tile.TileContext(nc) as tc, ExitStack() as top:
        # ---- constants ----
        const = top.enter_context(tc.tile_pool(name="const", bufs=1))
        e2 = const.tile([128, 2], BF16)
        nc.gpsimd.dma_start(out=e2, in_=ce2_d)
        b2 = const.tile([2, 128], BF16)
        nc.gpsimd.dma_start(out=b2, in_=cb2_d)
        ceps = const.tile([128, 2], F32)
        nc.gpsimd.dma_start(out=ceps, in_=ceps_d)
        eps_t = ceps[:, 0:1]
        eps64_t = ceps[:, 1:2]
        bpb = const.tile([128, C], BF16)
        nc.gpsimd.dma_start(out=bpb, in_=bpb_d)
        selc = const.tile([128, 128], BF16)
        nc.gpsimd.dma_start(out=selc, in_=csel_d)

        wpool = top.enter_context(tc.tile_pool(name="w", bufs=1))
        wts = {k: wpool.tile([128, CCH, C], BF16, name=f"w_{k}")
               for k in ("v", "q", "k", "p")}

        def w_dma_units(kinds):
            for k, dram in kinds:
                dr = dram.rearrange("(cc p) d -> p cc d", p=128)
                for cc in range(CCH):
                    def u(k=k, dr=dr, cc=cc):
                        nc.gpsimd.dma_start(out=wts[k][:, cc, :],
                                            in_=dr[:, cc, :])
                    yield u

        # ---- persistent tiles; yT/qkv lifetimes are disjoint across the two
        # batches so one set is shared; AO/rA overlap in P4 so two copies ----
        big = top.enter_context(tc.tile_pool(name="big", bufs=1))
        shared_tiles = dict(
            yT=big.tile([128, CCH, N], BF16, name="yT"),
            qT=big.tile([128, CCH, N], BF16, name="qT"),
            kT=big.tile([128, CCH, N], BF16, name="kT"),
        )
        bstate = []
        for b in range(BL):
            st = dict(shared_tiles)
            st["vS"] = big.tile([128, NB, H, Dh + 1], BF16, name=f"vS{b}")
            st["AO"] = big.tile([128, CCH, N], BF16, name=f"AO{b}")
            st["stg"] = big.tile([128, N], BF16, name=f"stg{b}")
            bstate.append(st)

        # ================= phase emitters =================

        def a1_units(b, ph, ph_inner=None):
            """Token centering y = x - mean(x) + DMA-transpose to yT. The
            pre-LN rstd is NOT applied to y: the per-head post-LN on q/k is
            scale-invariant per token (weights are centered), so only v needs
            it — folded into the v psum->SBUF copy as a per-partition scale.
            Act sqrts happen in one burst to avoid act-table thrash against
            the exp stream. Returns (part1s, [sqrt burst], rstd_fn)."""
            st = bstate[b]
            phi = ph_inner if ph_inner is not None else ph
            mp = ph.enter_context(tc.tile_pool(name=f"a1m{b}", bufs=1))
            mvAll = mp.tile([128, NB, nc.vector.BN_AGGR_DIM], F32,
                            name=f"mv{b}")
            rsAll = mp.tile([128, NB, 1], F32, name=f"rs{b}")
            xp = phi.enter_context(tc.tile_pool(name=f"a1x{b}", bufs=3))
            sp = phi.enter_context(tc.tile_pool(name=f"a1s{b}", bufs=3))
            yp = phi.enter_context(tc.tile_pool(name=f"a1y{b}", bufs=2))
            p1s = []
            for t in range(NB):
                def part1(t=t):
                    r0 = b * N + t * 128
                    xt = xp.tile([128, C], BF16, tag="xt")
                    nc.sync.dma_start(out=xt, in_=x_d[r0:r0 + 128, :])
                    stats = sp.tile([128, 2, nc.vector.BN_STATS_DIM], F32,
                                    tag="st")
                    xg = xt.rearrange("p (s f) -> p s f", s=2)
                    for s in range(2):
                        nc.vector.bn_stats(out=stats[:, s, :], in_=xg[:, s, :])
                    nc.vector.bn_aggr(out=mvAll[:, t, :], in_=stats)
                    y = yp.tile([128, C], BF16, tag="y")
                    nc.vector.tensor_scalar(
                        out=y, in0=xt, scalar1=mvAll[:, t, 0:1], scalar2=None,
                        op0=OP.subtract)
                    nc.scalar.dma_start(
                        out=st["yT"][:, :, t * 128:(t + 1) * 128], in_=y,
                        transpose=True)

                p1s.append(part1)

            def sqrt_burst():
                # single Act instruction (cannot be scheduler-split between
                # exps) + single DVE reciprocal for all NB tiles' rstd
                nc.scalar.activation(rsAll, mvAll[:, :, 1:2], AF.Sqrt,
                                     bias=eps_t)
                nc.vector.reciprocal_approx_fast(out=rsAll, in_=rsAll)

            def rstd_of(t):
                return rsAll[:, t, 0:1]

            return p1s, [sqrt_burst], rstd_of

        def v_units(b, pool, rstd_of):
            """v projection units (one per token tile), using psum `pool`.
            Applies the pre-LN rstd as a per-partition (per-token) scale."""
            st = bstate[b]
            units = []
            for tt in range(NB):
                def vunit(tt=tt):
                    for d2 in range(2):
                        ps = pool.tile([128, 512], F32, tag="ppv", name="ppv")
                        for cc in range(CCH):
                            nc.tensor.matmul(
                                ps, st["yT"][:, cc, tt * 128:(tt + 1) * 128],
                                wts["v"][:, cc, d2 * 512:(d2 + 1) * 512],
                                start=(cc == 0), stop=(cc == CCH - 1))
                        nc.vector.tensor_scalar(
                            out=st["vS"][:, tt, d2 * 8:(d2 + 1) * 8, 0:Dh],
                            in0=ps.rearrange("p (h e) -> p h e", e=Dh),
                            scalar1=rstd_of(tt), scalar2=None, op0=OP.mult)
                    nc.gpsimd.memset(st["vS"][:, tt, :, Dh:Dh + 1], 1.0)
                units.append(vunit)
            return units

        def pipeline_units(*stages):
            """Yield stage-s unit j at step j+s (software pipeline)."""
            n = max(len(s) for s in stages)
            for i in range(n + len(stages) - 1):
                for s, lst in enumerate(stages):
                    j = i - s
                    if 0 <= j < len(lst):
                        yield lst[j]

        def a2_units(b, ph, rstd_of=None):
            """q/k (+v when rstd_of given) projections, software-pipelined."""
            st = bstate[b]
            pp = ph.enter_context(tc.tile_pool(name=f"pp{b}", bufs=2,
                                               space="PSUM"))
            ppv = ph.enter_context(tc.tile_pool(name=f"pv{b}", bufs=1,
                                                space="PSUM"))
            sbq = ph.enter_context(tc.tile_pool(name=f"sq{b}", bufs=1,
                                                space="PSUM"))
            sb = ph.enter_context(tc.tile_pool(name=f"sb{b}", bufs=1,
                                               space="PSUM"))
            wkk = ph.enter_context(tc.tile_pool(name=f"wk{b}", bufs=2))
            wk2 = ph.enter_context(tc.tile_pool(name=f"w2{b}", bufs=1))

            if rstd_of is not None:
                for u in v_units(b, ppv, rstd_of):
                    yield u

            # q then k, per head-pair chunk dc; parts pipelined at depth 2
            specs = [(0, dc) for dc in range(CCH)] + \
                    [(1, dc) for dc in range(CCH)]
            states = [{} for _ in specs]

            def partA(u):
                wi, dc = specs[u]
                s = states[u]
                wt = wts["q"] if wi == 0 else wts["k"]
                raw = (st["qT"] if wi == 0 else st["kT"])[:, dc, :]
                ps = pp.tile([128, N], F32, tag="pp")
                for q2 in range(2):
                    sl = slice(q2 * 512, (q2 + 1) * 512)
                    for cc in range(CCH):
                        nc.tensor.matmul(
                            ps[:, sl], wt[:, cc, dc * 128:(dc + 1) * 128],
                            st["yT"][:, cc, sl],
                            start=(cc == 0), stop=(cc == CCH - 1))
                nc.vector.tensor_copy(out=raw, in_=ps)
                sq = wkk.tile([128, N], BF16, tag="sq")
                nc.vector.tensor_mul(sq, raw, raw)
                s["raw"], s["sq"] = raw, sq

            def partB(u):
                wi, dc = specs[u]
                s = states[u]
                ssq = sbq.tile([2, N], F32, tag="ssq")
                for q2 in range(2):
                    sl = slice(q2 * 512, (q2 + 1) * 512)
                    nc.tensor.matmul(ssq[:, sl], e2, s["sq"][:, sl],
                                     start=True, stop=True)
                stdt = wk2.tile([2, N], F32, tag="stdt")
                if wi == 0:
                    # 0.125/sqrt(ssq/64+eps) == 1/sqrt(ssq+64eps)
                    nc.scalar.activation(stdt, ssq, AF.Sqrt,
                                         bias=eps64_t[0:2, :])
                else:
                    nc.scalar.activation(stdt, ssq, AF.Sqrt,
                                         bias=eps_t[0:2, :], scale=1.0 / 64.0)
                rst = wk2.tile([2, N], BF16, tag="rst")
                with nc.allow_low_precision(reason="bf16 rstd"):
                    nc.vector.reciprocal(rst, stdt)
                s["rst"] = rst

            def partC(u):
                wi, dc = specs[u]
                s = states[u]
                for q2 in range(2):
                    sl = slice(q2 * 512, (q2 + 1) * 512)
                    bc = sb.tile([128, 512], F32, tag="bc", name="bc")
                    nc.tensor.matmul(bc, b2, s["rst"][:, sl],
                                     start=True, stop=True)
                    nc.vector.tensor_tensor(out=s["raw"][:, sl],
                                            in0=s["raw"][:, sl],
                                            in1=bc, op=OP.mult)

            nu = len(specs)
            for u in range(nu + 2):
                def unit(u=u):
                    if u < nu:
                        partA(u)
                    if 1 <= u < nu + 1:
                        partB(u - 1)
                    if 2 <= u:
                        partC(u - 2)
                yield unit


        def b_units(b, ph, o_bufs):
            """Attention stage: one flat software-pipelined stream over all
            (head, kc) steps — s/e of step i+1 always precede o of step i in
            program order, across head boundaries, so the exp stream never
            drains."""
            st = bstate[b]
            spool = ph.enter_context(tc.tile_pool(name=f"s{b}", bufs=2,
                                                  space="PSUM"))
            opool = ph.enter_context(tc.tile_pool(name=f"o{b}", bufs=o_bufs,
                                                  space="PSUM"))
            epool = ph.enter_context(tc.tile_pool(name=f"e{b}", bufs=2))
            bcp = ph.enter_context(tc.tile_pool(name=f"bc{b}", bufs=1,
                                                space="PSUM"))
            nc.gpsimd.memset(st["stg"][64:128, :], 0.0)
            steps = [(h, kc) for h in range(H) for kc in range(NB)]
            state = {}

            def se(i):
                h, kc = steps[i]
                hp, par = h // 2, h % 2
                s = spool.tile([128, N], F32, tag="s")
                for q2 in range(2):
                    sl = slice(q2 * 512, (q2 + 1) * 512)
                    nc.tensor.matmul(
                        s[:, sl], st["kT"][64 * par:64 * par + 64, hp,
                                           kc * 128:(kc + 1) * 128],
                        st["qT"][64 * par:64 * par + 64, hp, sl],
                        start=True, stop=True)
                e = epool.tile([128, N], BF16, tag="e")
                nc.scalar.activation(e, s, AF.Exp)
                state[i] = e

            def ov(i):
                h, kc = steps[i]
                hp, par = h // 2, h % 2
                if kc == 0:
                    opsum = opool.tile([Dh + 1, N], F32, tag="o", name="o")
                    state["o"] = opsum
                e = state.pop(i)
                for q2 in range(2):
                    sl = slice(q2 * 512, (q2 + 1) * 512)
                    nc.tensor.matmul(state["o"][:, sl],
                                     st["vS"][:, kc, h, :], e[:, sl],
                                     start=(kc == 0), stop=(kc == NB - 1))
                if kc == NB - 1:
                    opsum = state["o"]
                    # denominator reciprocal into stage row 64 (even head)
                    # or 96 (odd head) - 32-aligned partitions
                    row = 64 + 32 * par
                    with nc.allow_low_precision(reason="bf16 denom"):
                        nc.vector.reciprocal(st["stg"][row:row + 1, :],
                                             opsum[Dh:Dh + 1, :])
                    nc.vector.tensor_copy(
                        out=st["AO"][64 * par:64 * par + 64, hp, :],
                        in_=opsum[0:Dh, :])
                    if par == 1:
                        # normalize the completed head pair: broadcast the
                        # two recip rows to 128 partitions, scale AO in place
                        for q2 in range(2):
                            sl = slice(q2 * 512, (q2 + 1) * 512)
                            bc = bcp.tile([128, 512], F32, tag="bc",
                                          name="bc")
                            nc.tensor.matmul(
                                bc, selc[64:128, :], st["stg"][64:128, sl],
                                start=True, stop=True)
                            nc.vector.tensor_tensor(
                                out=st["AO"][:, hp, sl],
                                in0=st["AO"][:, hp, sl], in1=bc, op=OP.mult)

            ns = len(steps)
            for i in range(ns + 1):
                def unit(i=i):
                    if i < ns:
                        se(i)
                    if i >= 1:
                        ov(i - 1)
                yield unit

        def c_units(b, ph, bufs):
            """Output projection; one unit per (tt, d2)."""
            st = bstate[b]
            cps = ph.enter_context(tc.tile_pool(name=f"cp{b}", bufs=bufs,
                                                space="PSUM"))
            cop = ph.enter_context(tc.tile_pool(name=f"co{b}", bufs=3))
            for tt in range(NB):
                for d2 in range(2):
                    def unit(tt=tt, d2=d2):
                        ps = cps.tile([128, 512], F32, tag="cp")
                        for cc in range(CCH):
                            nc.tensor.matmul(
                                ps, st["AO"][:, cc, tt * 128:(tt + 1) * 128],
                                wts["p"][:, cc, d2 * 512:(d2 + 1) * 512],
                                start=(cc == 0), stop=(cc == CCH - 1))
                        osb = cop.tile([128, 512], BF16, tag="osb")
                        nc.vector.tensor_tensor(
                            out=osb, in0=ps,
                            in1=bpb[:, d2 * 512:(d2 + 1) * 512], op=OP.add)
                        nc.sync.dma_start(
                            out=out_d[b * N + tt * 128:b * N + (tt + 1) * 128,
                                      d2 * 512:(d2 + 1) * 512],
                            in_=osb)
                    yield unit

        def run_all(gen):
            for u in gen:
                u()

        def run_interleaved(main_gen, fill_gen, fill_per_main):
            fill_iter = iter(fill_gen)
            acc = 0.0
            for u in main_gen:
                u()
                acc += fill_per_main
                while acc >= 1.0:
                    acc -= 1.0
                    done = True
                    for f in fill_iter:
                        f()
                        done = False
                        break
                    if done:
                        acc = 0.0
            for f in fill_iter:
                f()

        # ================= schedule =================
        with ExitStack() as ph01:
            # P0: A1(b0), weight loads interleaved into the DMA queue
            with ExitStack() as ph0i:
                p1s, burst, rstd0 = a1_units(0, ph01, ph0i)
                run_interleaved(iter(p1s + burst),
                                w_dma_units([("v", wvt_d), ("q", wqt_d),
                                             ("k", wkt_d)]), 2.0)
                run_all(w_dma_units([("p", wpt_d)]))
            run_all(a2_units(0, ph01, rstd_of=rstd0))      # P1
        with ExitStack() as ph2:           # P2: B(0) || A1(1)+v(1)
            ppv = ph2.enter_context(tc.tile_pool(name="ppv", bufs=1,
                                                 space="PSUM"))
            p1s, burst, rstd1 = a1_units(1, ph2)
            fills = p1s + burst + v_units(1, ppv, rstd1)
            run_interleaved(b_units(0, ph2, o_bufs=1), iter(fills), 0.14)
        with ExitStack() as ph3:
            run_all(a2_units(1, ph3))                      # P3
        with ExitStack() as ph4:           # P4: B(1) || C(0)
            run_interleaved(b_units(1, ph4, o_bufs=1),
                            iter(c_units(0, ph4, bufs=1)), 0.13)
        with ExitStack() as ph5:           # P5: C(1)
            run_all(c_units(1, ph5, bufs=2))

    nc.compile()
    return nc


def _get_nc():
    if "nc" not in _cache:
        _cache["nc"] = _build()
    return _cache["nc"]


def _host_inputs(Wq, Wk, Wv, Wp, bp):
    """Shared (core-independent) derived weight tensors."""
    import ml_dtypes
    bf16 = ml_dtypes.bfloat16

    def center(Wm):
        Wh = np.asarray(Wm, dtype=np.float32).reshape(H, Dh, C)
        return (Wh - Wh.mean(axis=1, keepdims=True)).reshape(C, C)

    e2 = np.zeros((128, 2), np.float32)
    e2[0:64, 0] = 1.0
    e2[64:128, 1] = 1.0
    b2 = np.zeros((2, 128), np.float32)
    b2[0, 0:64] = 1.0
    b2[1, 64:128] = 1.0
    eps = np.zeros((128, 2), np.float32)
    eps[:, 0] = EPS
    eps[:, 1] = 64.0 * EPS
    bpb = np.broadcast_to(np.asarray(bp, np.float32).reshape(1, C),
                          (128, C)).astype(bf16)
    sel = np.zeros((128, 128), np.float32)
    sel[64, 0:64] = 1.0
    sel[96, 64:128] = 1.0
    return {
        "c_sel": sel.astype(bf16),
        "c_e2": e2.astype(bf16),
        "c_b2": b2.astype(bf16),
        "c_eps": eps,
        "bpb": bpb,
        "wqt": np.ascontiguousarray(center(Wq).T).astype(bf16),
        "wkt": np.ascontiguousarray(center(Wk).T).astype(bf16),
        "wvt": np.ascontiguousarray(np.asarray(Wv, np.float32).T).astype(bf16),
        "wpt": np.ascontiguousarray(np.asarray(Wp, np.float32).T).astype(bf16),
    }


def _in_maps(x, Wq, Wk, Wv, Wp, bp):
    import ml_dtypes

    shared = _host_inputs(Wq, Wk, Wv, Wp, bp)
    xbf = np.asarray(x, dtype=np.float32).astype(ml_dtypes.bfloat16)
    return [
        dict(shared,
             xbf=np.ascontiguousarray(xbf[c * BL:(c + 1) * BL].reshape(T, C)))
        for c in range(NCORES)
    ]


def kernel(x, Wq, Wk, Wv, Wp, bp):
    from concourse.bass_utils import run_bass_kernel_spmd

    nc = _get_nc()
    in_maps = _in_maps(x, Wq, Wk, Wv, Wp, bp)
    res = run_bass_kernel_spmd(nc, in_maps, core_ids=list(range(NCORES)))
    out = np.stack([res.results[c]["out"].reshape(BL, N, C)
                    for c in range(NCORES)])
    return out.reshape(B, N, C).astype(np.float32)


# revision 57
# speedup vs baseline: 1.3016x; 1.2874x over previous
"""Trainium2 Bass kernel for nn_Attention (B=16, N=1024, C=1024, H=16, pre-LN +
q/k post-LN attention block), data-parallel over 8 NeuronCores (2 batches/core).

v2 design (vs v1 baseline):
  - all-bf16 datapath (weights, y, q, k, v, e, AO, out); fp32 PSUM accumulate;
    host up-casts the output to f32 (halves the output transfer).
  - weights loaded into SBUF once per kernel (not per batch), in 8 chunk DMAs
    each on the gpsimd SWDGE queue so the SP queue stays free for x tiles.
  - pre-LN computes only y = x - mean: the per-head post-LN on q/k is
    scale-invariant per token (weights are centered), so the pre-LN rstd is
    folded solely into the v copy as a per-partition scale.
  - y transposed to yT via DMA xbar transposes on the Act HWDGE queue
    (no PE/PSUM involved).
  - exp tiles are merged [128 kt, 1024 qt] (one Act op per (head, kc)).
  - q/k post-LN scales via broadcast matmul + in-place multiply (q folds the
    1/8 attention scale via the 64*eps trick). All A1 sqrts batched into one
    Act instruction to avoid exp<->sqrt act-table thrashing.
  - softmax normalization per head pair inside stage B: denominator
    reciprocals land on 32-aligned stage rows (64/96), one selector matmul
    broadcasts both, one in-place AO multiply per half. Bias added by DVE
    from a broadcast tile (no bias matmuls).
  - software pipeline: B(b0) overlaps A1(b1)+v(b1) + weight loads; B(b1)
    overlaps C(b0); stage B is one flat (head, kc) software-pipelined stream.
"""

import numpy as np

B, N, C, H, Dh = 16, 1024, 1024, 16, 64
NCORES = 8
BL = B // NCORES          # batches per core
T = BL * N                # tokens per core
CCH = C // 128            # contraction chunks
NB = N // 128             # token tiles per batch
EPS = 1e-6

_cache: dict = {}


def _build():
    from contextlib import ExitStack

    import concourse.bacc as bacc
    import concourse.mybir as mybir
    import concourse.tile as tile

    F32 = mybir.dt.float32
    F32R = mybir.dt.float32r
    BF16 = mybir.dt.bfloat16
    AF = mybir.ActivationFunctionType
    OP = mybir.AluOpType

    nc = bacc.Bacc("TRN2", target_bir_lowering=False, debug=False,
                   num_devices=NCORES)

    x_d = nc.dram_tensor("xbf", [T, C], BF16, kind="ExternalInput").ap()
    wqt_d = nc.dram_tensor("wqt", [C, C], BF16, kind="ExternalInput").ap()
    wkt_d = nc.dram_tensor("wkt", [C, C], BF16, kind="ExternalInput").ap()
    wvt_d = nc.dram_tensor("wvt", [C, C], BF16, kind="ExternalInput").ap()
    wpt_d = nc.dram_tensor("wpt", [C, C], BF16, kind="ExternalInput").ap()
    bpb_d = nc.dram_tensor("bpb", [128, C], BF16, kind="ExternalInput").ap()
    csel_d = nc.dram_tensor("c_sel", [128, 128], BF16,
                            kind="ExternalInput").ap()
    ce2_d = nc.dram_tensor("c_e2", [128, 2], BF16, kind="ExternalInput").ap()
    cb2_d = nc.dram_tensor("c_b2", [2, 128], BF16, kind="ExternalInput").ap()
    ceps_d = nc.dram_tensor("c_eps", [128, 2], F32, kind="ExternalInput").ap()
    out_d = nc.dram_tensor("out", [T, C], BF16, kind="ExternalOutput").ap()

    with tile.TileContext(nc) as tc, ExitStack() as top:
        # ---- constants ----
        const = top.enter_context(tc.tile_pool(name="const", bufs=1))
        e2 = const.tile([128, 2], BF16)
        nc.gpsimd.dma_start(out=e2, in_=ce2_d)
        b2 = const.tile([2, 128], BF16)
        nc.gpsimd.dma_start(out=b2, in_=cb2_d)
        ceps = const.tile([128, 2], F32)
        nc.gpsimd.dma_start(out=ceps, in_=ceps_d)
        eps_t = ceps[:, 0:1]
        eps64_t = ceps[:, 1:2]
        bpb = const.tile([128, C], BF16)
        nc.gpsimd.dma_start(out=bpb, in_=bpb_d)
        selc = const.tile([128, 128], BF16)
        nc.gpsimd.dma_start(out=selc, in_=csel_d)

        wpool = top.enter_context(tc.tile_pool(name="w", bufs=1))
        wts = {k: wpool.tile([128, CCH, C], BF16, name=f"w_{k}")
               for k in ("v", "q", "k", "p")}

        def w_dma_units(kinds):
            for k, dram in kinds:
                dr = dram.rearrange("(cc p) d -> p cc d", p=128)
                for cc in range(CCH):
                    def u(k=k, dr=dr, cc=cc):
                        nc.gpsimd.dma_start(out=wts[k][:, cc, :],
                                            in_=dr[:, cc, :])
                    yield u

        # ---- persistent tiles; yT/qkv lifetimes are disjoint across the two
        # batches so one set is shared; AO/rA overlap in P4 so two copies ----
        big = top.enter_context(tc.tile_pool(name="big", bufs=1))
        shared_tiles = dict(
            yT=big.tile([128, CCH, N], BF16, name="yT"),
            qT=big.tile([128, CCH, N], BF16, name="qT"),
            kT=big.tile([128, CCH, N], BF16, name="kT"),
        )
        bstate = []
        for b in range(BL):
            st = dict(shared_tiles)
            st["vS"] = big.tile([128, NB, H, Dh + 1], BF16, name=f"vS{b}")
            st["AO"] = big.tile([128, CCH, N], BF16, name=f"AO{b}")
            st["stg"] = big.tile([128, N], BF16, name=f"stg{b}")
            bstate.append(st)

        # ================= phase emitters =================

        def a1_units(b, ph, ph_inner=None):
            """Token centering y = x - mean(x) + DMA-transpose to yT. The
            pre-LN rstd is NOT applied to y: the per-head post-LN on q/k is
            scale-invariant per token (weights are centered), so only v needs
            it — folded into the v psum->SBUF copy as a per-partition scale.
            Act sqrts happen in one burst to avoid act-table thrash against
            the exp stream. Returns (part1s, [sqrt burst], rstd_fn)."""
            st = bstate[b]
            phi = ph_inner if ph_inner is not None else ph
            mp = ph.enter_context(tc.tile_pool(name=f"a1m{b}", bufs=1))
            mvAll = mp.tile([128, NB, nc.vector.BN_AGGR_DIM], F32,
                            name=f"mv{b}")
            rsAll = mp.tile([128, NB, 1], F32, name=f"rs{b}")
            xp = phi.enter_context(tc.tile_pool(name=f"a1x{b}", bufs=3))
            sp = phi.enter_context(tc.tile_pool(name=f"a1s{b}", bufs=3))
            yp = phi.enter_context(tc.tile_pool(name=f"a1y{b}", bufs=2))
            p1s = []
            for t in range(NB):
                def part1(t=t):
                    r0 = b * N + t * 128
                    xt = xp.tile([128, C], BF16, tag="xt")
                    nc.sync.dma_start(out=xt, in_=x_d[r0:r0 + 128, :])
                    stats = sp.tile([128, 2, nc.vector.BN_STATS_DIM], F32,
                                    tag="st")
                    xg = xt.rearrange("p (s f) -> p s f", s=2)
                    for s in range(2):
                        nc.vector.bn_stats(out=stats[:, s, :], in_=xg[:, s, :])
                    nc.vector.bn_aggr(out=mvAll[:, t, :], in_=stats)
                    y = yp.tile([128, C], BF16, tag="y")
                    nc.vector.tensor_scalar(
                        out=y, in0=xt, scalar1=mvAll[:, t, 0:1], scalar2=None,
                        op0=OP.subtract)
                    nc.scalar.dma_start(
                        out=st["yT"][:, :, t * 128:(t + 1) * 128], in_=y,
                        transpose=True)

                p1s.append(part1)

            def sqrt_burst():
                # single Act instruction (cannot be scheduler-split between
                # exps) + single DVE reciprocal for all NB tiles' rstd
                nc.scalar.activation(rsAll, mvAll[:, :, 1:2], AF.Sqrt,
                                     bias=eps_t)
                nc.vector.reciprocal_approx_fast(out=rsAll, in_=rsAll)

            def rstd_of(t):
                return rsAll[:, t, 0:1]

            return p1s, [sqrt_burst], rstd_of

        def v_units(b, pool, rstd_of):
            """v projection units (one per token tile), using psum `pool`.
            Applies the pre-LN rstd as a per-partition (per-token) scale."""
            st = bstate[b]
            units = []
            for tt in range(NB):
                def vunit(tt=tt):
                    for d2 in range(2):
                        ps = pool.tile([128, 512], F32, tag="ppv", name="ppv")
                        for cc in range(CCH):
                            nc.tensor.matmul(
                                ps, st["yT"][:, cc, tt * 128:(tt + 1) * 128],
                                wts["v"][:, cc, d2 * 512:(d2 + 1) * 512],
                                start=(cc == 0), stop=(cc == CCH - 1))
                        nc.vector.tensor_scalar(
                            out=st["vS"][:, tt, d2 * 8:(d2 + 1) * 8, 0:Dh],
                            in0=ps.rearrange("p (h e) -> p h e", e=Dh),
                            scalar1=rstd_of(tt), scalar2=None, op0=OP.mult)
                    nc.gpsimd.memset(st["vS"][:, tt, :, Dh:Dh + 1], 1.0)
                units.append(vunit)
            return units

        def pipeline_units(*stages):
            """Yield stage-s unit j at step j+s (software pipeline)."""
            n = max(len(s) for s in stages)
            for i in range(n + len(stages) - 1):
                for s, lst in enumerate(stages):
                    j = i - s
                    if 0 <= j < len(lst):
                        yield lst[j]

        def a2_units(b, ph, rstd_of=None):
            """q/k (+v when rstd_of given) projections, software-pipelined."""
            st = bstate[b]
            pp = ph.enter_context(tc.tile_pool(name=f"pp{b}", bufs=2,
                                               space="PSUM"))
            ppv = ph.enter_context(tc.tile_pool(name=f"pv{b}", bufs=1,
                                                space="PSUM"))
            sbq = ph.enter_context(tc.tile_pool(name=f"sq{b}", bufs=1,
                                                space="PSUM"))
            sb = ph.enter_context(tc.tile_pool(name=f"sb{b}", bufs=1,
                                               space="PSUM"))
            wkk = ph.enter_context(tc.tile_pool(name=f"wk{b}", bufs=2))
            wk2 = ph.enter_context(tc.tile_pool(name=f"w2{b}", bufs=1))

            if rstd_of is not None:
                for u in v_units(b, ppv, rstd_of):
                    yield u

            # q then k, per head-pair chunk dc; parts pipelined at depth 2
            specs = [(0, dc) for dc in range(CCH)] + \
                    [(1, dc) for dc in range(CCH)]
            states = [{} for _ in specs]

            def partA(u):
                wi, dc = specs[u]
                s = states[u]
                wt = wts["q"] if wi == 0 else wts["k"]
                raw = (st["qT"] if wi == 0 else st["kT"])[:, dc, :]
                ps = pp.tile([128, N], F32, tag="pp")
                for q2 in range(2):
                    sl = slice(q2 * 512, (q2 + 1) * 512)
                    for cc in range(CCH):
                        nc.tensor.matmul(
                            ps[:, sl], wt[:, cc, dc * 128:(dc + 1) * 128],
                            st["yT"][:, cc, sl],
                            start=(cc == 0), stop=(cc == CCH - 1))
                nc.vector.tensor_copy(out=raw, in_=ps)
                sq = wkk.tile([128, N], BF16, tag="sq")
                nc.vector.tensor_mul(sq, raw, raw)
                s["raw"], s["sq"] = raw, sq

            def partB(u):
                wi, dc = specs[u]
                s = states[u]
                ssq = sbq.tile([2, N], F32, tag="ssq")
                for q2 in range(2):
                    sl = slice(q2 * 512, (q2 + 1) * 512)
                    nc.tensor.matmul(ssq[:, sl], e2, s["sq"][:, sl],
                                     start=True, stop=True)
                stdt = wk2.tile([2, N], F32, tag="stdt")
                if wi == 0:
                    # 0.125/sqrt(ssq/64+eps) == 1/sqrt(ssq+64eps)
                    nc.scalar.activation(stdt, ssq, AF.Sqrt,
                                         bias=eps64_t[0:2, :])
                else:
                    nc.scalar.activation(stdt, ssq, AF.Sqrt,
                                         bias=eps_t[0:2, :], scale=1.0 / 64.0)
                rst = wk2.tile([2, N], BF16, tag="rst")
                with nc.allow_low_precision(reason="bf16 rstd"):
                    nc.vector.reciprocal(rst, stdt)
                s["rst"] = rst

            def partC(u):
                wi, dc = specs[u]
                s = states[u]
                for q2 in range(2):
                    sl = slice(q2 * 512, (q2 + 1) * 512)
                    bc = sb.tile([128, 512], F32, tag="bc", name="bc")
                    nc.tensor.matmul(bc, b2, s["rst"][:, sl],
                                     start=True, stop=True)
                    nc.vector.tensor_tensor(out=s["raw"][:, sl],
                                            in0=s["raw"][:, sl],
                                            in1=bc, op=OP.mult)

            nu = len(specs)
            for u in range(nu + 2):
                def unit(u=u):
                    if u < nu:
                        partA(u)
                    if 1 <= u < nu + 1:
                        partB(u - 1)
                    if 2 <= u:
                        partC(u - 2)
                yield unit


        def b_units(b, ph, o_bufs):
            """Attention stage: one flat software-pipelined stream over all
            (head, kc) steps — s/e of step i+1 always precede o of step i in
            program order, across head boundaries, so the exp stream never
            drains."""
            st = bstate[b]
            spool = ph.enter_context(tc.tile_pool(name=f"s{b}", bufs=2,
                                                  space="PSUM"))
            opool = ph.enter_context(tc.tile_pool(name=f"o{b}", bufs=o_bufs,
                                                  space="PSUM"))
            epool = ph.enter_context(tc.tile_pool(name=f"e{b}", bufs=2))
            bcp = ph.enter_context(tc.tile_pool(name=f"bc{b}", bufs=1,
                                                space="PSUM"))
            nc.gpsimd.memset(st["stg"][64:128, :], 0.0)
            steps = [(h, kc) for h in range(H) for kc in range(NB)]
            state = {}

            def se(i):
                h, kc = steps[i]
                hp, par = h // 2, h % 2
                s = spool.tile([128, N], F32, tag="s")
                for q2 in range(2):
                    sl = slice(q2 * 512, (q2 + 1) * 512)
                    nc.tensor.matmul(
                        s[:, sl], st["kT"][64 * par:64 * par + 64, hp,
                                           kc * 128:(kc + 1) * 128],
                        st["qT"][64 * par:64 * par + 64, hp, sl],
                        start=True, stop=True)
                e = epool.tile([128, N], BF16, tag="e")
                nc.scalar.activation(e, s, AF.Exp)
                state[i] = e

            def ov(i):
                h, kc = steps[i]
                hp, par = h // 2, h % 2
                if kc == 0:
                    opsum = opool.tile([Dh + 1, N], F32, tag="o", name="o")
                    state["o"] = opsum
                e = state.pop(i)
                for q2 in range(2):
                    sl = slice(q2 * 512, (q2 + 1) * 512)
                    nc.tensor.matmul(state["o"][:, sl],
                                     st["vS"][:, kc, h, :], e[:, sl],
                                     start=(kc == 0), stop=(kc == NB - 1))
                if kc == NB - 1:
                    opsum = state["o"]
                    # denominator reciprocal into stage row 64 (even head)
                    # or 96 (odd head) - 32-aligned partitions
                    row = 64 + 32 * par
                    with nc.allow_low_precision(reason="bf16 denom"):
                        nc.vector.reciprocal(st["stg"][row:row + 1, :],
                                             opsum[Dh:Dh + 1, :])
                    nc.vector.tensor_copy(
                        out=st["AO"][64 * par:64 * par + 64, hp, :],
                        in_=opsum[0:Dh, :])
                    if par == 1:
                        # normalize the completed head pair: broadcast the
                        # two recip rows to 128 partitions, scale AO in place
                        for q2 in range(2):
                            sl = slice(q2 * 512, (q2 + 1) * 512)
                            bc = bcp.tile([128, 512], F32, tag="bc",
                                          name="bc")
                            nc.tensor.matmul(
                                bc, selc[64:128, :], st["stg"][64:128, sl],
                                start=True, stop=True)
                            nc.vector.tensor_tensor(
                                out=st["AO"][:, hp, sl],
                                in0=st["AO"][:, hp, sl], in1=bc, op=OP.mult)

            ns = len(steps)
            for i in range(ns + 1):
                def unit(i=i):
                    if i < ns:
                        se(i)
                    if i >= 1:
                        ov(i - 1)
                yield unit

        def c_units(b, ph, bufs):
            """Output projection; one unit per (tt, d2)."""
            st = bstate[b]
            cps = ph.enter_context(tc.tile_pool(name=f"cp{b}", bufs=bufs,
                                                space="PSUM"))
            cop = ph.enter_context(tc.tile_pool(name=f"co{b}", bufs=3))
            for tt in range(NB):
                for d2 in range(2):
                    def unit(tt=tt, d2=d2):
                        ps = cps.tile([128, 512], F32, tag="cp")
                        for cc in range(CCH):
                            nc.tensor.matmul(
                                ps, st["AO"][:, cc, tt * 128:(tt + 1) * 128],
                                wts["p"][:, cc, d2 * 512:(d2 + 1) * 512],
                                start=(cc == 0), stop=(cc == CCH - 1))
                        osb = cop.tile([128, 512], BF16, tag="osb")
                        nc.vector.tensor_tensor(
                            out=osb, in0=ps,
                            in1=bpb[:, d2 * 512:(d2 + 1) * 512], op=OP.add)
                        nc.sync.dma_start(
                            out=out_d[b * N + tt * 128:b * N + (tt + 1) * 128,
                                      d2 * 512:(d2 + 1) * 512],
                            in_=osb)
                    yield unit

        def run_all(gen):
            for u in gen:
                u()

        def run_interleaved(main_gen, fill_gen, fill_per_main):
            fill_iter = iter(fill_gen)
            acc = 0.0
            for u in main_gen:
                u()
                acc += fill_per_main
                while acc >= 1.0:
                    acc -= 1.0
                    done = True
                    for f in fill_iter:
                        f()
                        done = False
                        break
                    if done:
                        acc = 0.0
            for f in fill_iter:
                f()

        # ================= schedule =================
        with ExitStack() as ph01:
            # P0: A1(b0), weight loads interleaved into the DMA queue
            with ExitStack() as ph0i:
                p1s, burst, rstd0 = a1_units(0, ph01, ph0i)
                run_interleaved(iter(p1s + burst),
                                w_dma_units([("v", wvt_d), ("q", wqt_d),
                                             ("k", wkt_d)]), 2.0)
                run_all(w_dma_units([("p", wpt_d)]))
            run_all(a2_units(0, ph01, rstd_of=rstd0))      # P1
        with ExitStack() as ph2:           # P2: B(0) || A1(1)+v(1)
            ppv = ph2.enter_context(tc.tile_pool(name="ppv", bufs=1,
                                                 space="PSUM"))
            p1s, burst, rstd1 = a1_units(1, ph2)
            fills = p1s + burst + v_units(1, ppv, rstd1)
            run_interleaved(b_units(0, ph2, o_bufs=1), iter(fills), 0.14)
        with ExitStack() as ph3:
            run_all(a2_units(1, ph3))                      # P3
        with ExitStack() as ph4:           # P4: B(1) || C(0)
            run_interleaved(b_units(1, ph4, o_bufs=1),
                            iter(c_units(0, ph4, bufs=1)), 0.13)
        with ExitStack() as ph5:           # P5: C(1)
            run_all(c_units(1, ph5, bufs=2))

    nc.compile()
    return nc


def _get_nc():
    if "nc" not in _cache:
        _cache["nc"] = _build()
    return _cache["nc"]


def _host_inputs(Wq, Wk, Wv, Wp, bp):
    """Shared (core-independent) derived weight tensors."""
    import ml_dtypes
    bf16 = ml_dtypes.bfloat16

    def center(Wm):
        Wh = np.asarray(Wm, dtype=np.float32).reshape(H, Dh, C)
        return (Wh - Wh.mean(axis=1, keepdims=True)).reshape(C, C)

    e2 = np.zeros((128, 2), np.float32)
    e2[0:64, 0] = 1.0
    e2[64:128, 1] = 1.0
    b2 = np.zeros((2, 128), np.float32)
    b2[0, 0:64] = 1.0
    b2[1, 64:128] = 1.0
    eps = np.zeros((128, 2), np.float32)
    eps[:, 0] = EPS
    eps[:, 1] = 64.0 * EPS
    bpb = np.broadcast_to(np.asarray(bp, np.float32).reshape(1, C),
                          (128, C)).astype(bf16)
    sel = np.zeros((128, 128), np.float32)
    sel[64, 0:64] = 1.0
    sel[96, 64:128] = 1.0
    return {
        "c_sel": sel.astype(bf16),
        "c_e2": e2.astype(bf16),
        "c_b2": b2.astype(bf16),
        "c_eps": eps,
        "bpb": bpb,
        "wqt": np.ascontiguousarray(center(Wq).T).astype(bf16),
        "wkt": np.ascontiguousarray(center(Wk).T).astype(bf16),
        "wvt": np.ascontiguousarray(np.asarray(Wv, np.float32).T).astype(bf16),
        "wpt": np.ascontiguousarray(np.asarray(Wp, np.float32).T).astype(bf16),
    }


def _in_maps(x, Wq, Wk, Wv, Wp, bp):
    import ml_dtypes

    shared = _host_inputs(Wq, Wk, Wv, Wp, bp)
    xbf = np.asarray(x, dtype=np.float32).astype(ml_dtypes.bfloat16)
    return [
        dict(shared,
             xbf=np.ascontiguousarray(xbf[c * BL:(c + 1) * BL].reshape(T, C)))
        for c in range(NCORES)
    ]


def kernel(x, Wq, Wk, Wv, Wp, bp):
    from concourse.bass_utils import run_bass_kernel_spmd

    nc = _get_nc()
    in_maps = _in_maps(x, Wq, Wk, Wv, Wp, bp)
    res = run_bass_kernel_spmd(nc, in_maps, core_ids=list(range(NCORES)))
    out = np.stack([res.results[c]["out"].reshape(BL, N, C)
                    for c in range(NCORES)])
    return out.reshape(B, N, C).astype(np.float32)
